# revision 11
# baseline (speedup 1.0000x reference)
"""Trainium2 Bass kernel for nn_CustomLoss_45449343926664 (retrieval_knn).

loss = mse(mean(c1), mean(c2))
     + mean_i min_j ||c1_i - c2_j||^2
     + mean_k relu(0.1 - var(c1)_k)

Device computes the dominant term: per-row max_j(2<c1_i,c2_j> - |c2_j|^2)
(min-distance via d2 = |c1_i|^2 - that max). The tiny O(N*D) stats
(means / variances / |c1_i|^2) are host-side in fp64, fused into the
final scalar in _finish.

Sharding (8 cores = 4 i-groups x 2 j-halves): core c owns c1 rows
[2048*(c%4), 2048*(c%4+1)) and c2 rows [4096*(c//4), 4096*(c//4+1)).

Per core: 32 j-tiles of 128, each computed as two [128 j, 1024 i] PSUM
units (2 banks each, psum pool bufs=4 so the PE runs ahead of the
drains). Cross matmuls in "j-on-partitions" orientation (c2bT tile
stationary, c1bT moving, bf16, c1 pre-scaled by 2). The 64 units drain
through the only two engines with PSUM read ports, balanced to ~equal
busy time:

  - 17 units: DVE fused scalar_tensor_tensor drain
        zD' = max(psum + bias_j, zD)       (1 pass, per-i-half ping-pong)
  - 47 units: ACT activation(Identity, bias_j) -> bf16 z tiles; pairs
    of same-i-half z tiles fold via one DVE bf16 tensor_tensor max (2x
    mode, [128, 2048]) into per-half ping-pong accumulators.

Tail (per i-half, pipelined): max(accA halves) -> max(.., zD) -> 8 PE
transposes -> 3D reduce_max -> gmax[p, b] for query i = 128*b + p
(+ 2048*(c%4)). Host combines the two j-halves and finishes in fp64.
"""
import os
import sys

import numpy as np
import ml_dtypes

if os.path.isdir("/opt/trn_rl_repo") and "/opt/trn_rl_repo" not in sys.path:
    sys.path.insert(0, "/opt/trn_rl_repo")

from contextlib import ExitStack

import concourse.bass as bass
import concourse.tile as tile
from concourse import bacc, mybir
from concourse.bass_utils import run_bass_kernel_spmd
from concourse.masks import make_identity

F32 = mybir.dt.float32
BF16 = mybir.dt.bfloat16
BF16_NP = ml_dtypes.bfloat16
NEG_BIG = -3.0e38

N_CORES = 8
N1 = 8192            # cluster1 rows (total)
N2 = 8192            # cluster2 rows
D = 128              # feature dim = partition count
P = 128
I_GROUPS = 4
J_HALVES = 2
NI = N1 // I_GROUPS  # 2048 c1 rows per core
NJ = N2 // J_HALVES  # 4096 c2 rows per core
NJT = NJ // P        # 32 j-tiles of 128
MTI = NI // P        # 16 i-blocks of 128 (for the transpose tail)
FDI = 1024           # i-extent per PSUM unit (2 banks)
NU = NJT * 2         # 64 drain units (j-tile x i-half)
MM_SPLIT = 2         # matmuls per unit (one PSUM bank each)

# units on the zD path (18 of 64, none in the final stretch); the first
# of each i-half is a seed executed on ACT (Identity+bias straight into
# zD), so DVE runs 16 fused stt drains and ACT 48 activations.
DVE_UNITS = frozenset((k * 56) // 18 for k in range(18))
MIN_VARIANCE = 0.1

_cached = {}


def _build_program():
    """Build + compile the single-core SPMD program (same for all cores)."""
    nc = bacc.Bacc(
        "TRN2",
        target_bir_lowering=False,
        debug=False,
        enable_asserts=False,
        num_devices=N_CORES,
    )

    d_c1bT = nc.dram_tensor("c1bT", [D, NI], BF16, kind="ExternalInput").ap()
    d_c2bT = nc.dram_tensor("c2bT", [D, NJ], BF16, kind="ExternalInput").ap()
    d_sq2neg = nc.dram_tensor("sq2neg", [P, NJT], F32, kind="ExternalInput").ap()

    d_gmax = nc.dram_tensor("gmax", [P, MTI], F32, kind="ExternalOutput").ap()

    with tile.TileContext(nc) as tc, ExitStack() as ctx:
        const = ctx.enter_context(tc.tile_pool(name="const", bufs=1))
        zpool = [ctx.enter_context(tc.tile_pool(name=f"zp{h}", bufs=3))
                 for h in range(2)]
        psum = ctx.enter_context(tc.tile_pool(name="psum", bufs=4, space="PSUM"))

        t_c1bT = const.tile([P, NI], BF16)
        t_c2bT = const.tile([P, NJ], BF16)
        t_sq2neg = const.tile([P, NJT], F32)
        # per-i-half fold accumulators (ping-pong) + DVE-direct accumulators
        t_zA = [[const.tile([P, 2, FDI], BF16, name=f"zA{h}_{i}")
                 for i in range(2)] for h in range(2)]
        t_zD = [const.tile([P, NI], BF16, name=f"zD{i}") for i in range(2)]
        t_zfin = const.tile([P, NI], BF16)
        t_gmax = const.tile([P, MTI], F32)
        t_ident = const.tile([P, P], BF16)
        t_dummy = const.tile([P, 1], F32)

        # ---- input DMAs: ALL on the sync ring, strict FIFO priority.
        # (All rings share the 16 SDMA engines packet-round-robin, so a
        # second ring's bulk load would steal bandwidth from the critical
        # head; one ring with careful ordering is strictly better.)
        nc.sync.dma_start(t_c2bT[:, 0 : 2 * P], d_c2bT[:, 0 : 2 * P])
        nc.sync.dma_start(t_c1bT[:, 0:512], d_c1bT[:, 0:512])
        nc.sync.dma_start(t_c1bT[:, 512:FDI], d_c1bT[:, 512:FDI])
        nc.sync.dma_start(t_sq2neg[:], d_sq2neg)
        nc.sync.dma_start(t_c2bT[:, 2 * P : 8 * P], d_c2bT[:, 2 * P : 8 * P])
        nc.sync.dma_start(t_c1bT[:, FDI:], d_c1bT[:, FDI:])
        nc.sync.dma_start(t_c2bT[:, 8 * P : 20 * P], d_c2bT[:, 8 * P : 20 * P])
        nc.sync.dma_start(t_c2bT[:, 20 * P :], d_c2bT[:, 20 * P :])

        # warm the ACT function table (load ~1.3us) before the first drain
        nc.vector.memset(t_dummy[:], 1.0)
        nc.scalar.activation(t_dummy[:], t_dummy[:],
                             mybir.ActivationFunctionType.Identity, bias=0.0)

        # identity (for the PE transpose tail) on gpsimd
        make_identity(nc, t_ident[:])

        # ramp the PE p-state while the first inputs land (takes a pool slot
        # whose WAW release happens naturally when the ring wraps)
        pwarm = psum.tile([P, FDI], F32, tag="pcross", name="pwarm")
        for w in range(6):
            nc.tensor.matmul(pwarm[:, :P], t_ident[:], t_ident[:],
                             start=(w == 0), stop=(w == 5))

        # ---- cross matmuls (j on partitions) + dual-engine drain ----
        nd = [0, 0]          # zD ping-pong index per i-half
        na = [0, 0]          # zA ping-pong index per i-half
        zpend = [None, None]  # partially-filled z pair per i-half
        seq = [(t, 0) for t in range(NJT)] + [(t, 1) for t in range(NJT)]
        for u, (t, h) in enumerate(seq):
            pt = psum.tile([P, FDI], F32, tag="pcross", name="pcross")
            lhsT = t_c2bT[:, t * P : (t + 1) * P]
            nmm = MM_SPLIT
            fd = FDI // nmm
            for c in range(nmm):
                nc.tensor.matmul(
                    pt[:, c * fd : (c + 1) * fd],
                    lhsT,
                    t_c1bT[:, h * FDI + c * fd : h * FDI + (c + 1) * fd],
                    start=True,
                    stop=True,
                )
            bias = t_sq2neg[:, t : t + 1]
            if u in DVE_UNITS:
                io = h * FDI
                if nd[h] == 0:
                    # first unit of this half seeds the accumulator straight
                    # from PSUM on the ACT engine (Identity + bias)
                    nc.scalar.activation(
                        t_zD[1][:, io : io + FDI], pt[:],
                        mybir.ActivationFunctionType.Identity,
                        bias=bias, scale=1.0)
                else:
                    nc.vector.scalar_tensor_tensor(
                        out=t_zD[(nd[h] + 1) % 2][:, io : io + FDI],
                        in0=pt[:],
                        scalar=bias,
                        in1=t_zD[nd[h] % 2][:, io : io + FDI],
                        op0=mybir.AluOpType.add,
                        op1=mybir.AluOpType.max,
                    )
                nd[h] += 1
            else:
                if zpend[h] is None:
                    zpend[h] = (zpool[h].tile([P, 2, FDI], BF16, name=f"z{h}"), 0)
                zt, m = zpend[h]
                nc.scalar.activation(
                    zt[:, m], pt[:], mybir.ActivationFunctionType.Identity,
                    bias=bias, scale=1.0,
                )
                if m == 1:
                    # one bf16 2x tensor_max folds the pair into the two
                    # independent slots of this half's accumulator
                    if na[h] == 0:
                        nc.vector.tensor_copy(t_zA[h][1][:], zt[:])
                    else:
                        nc.vector.tensor_max(t_zA[h][(na[h] + 1) % 2][:],
                                             t_zA[h][na[h] % 2][:], zt[:])
                    na[h] += 1
                    zpend[h] = None
                else:
                    zpend[h] = (zt, 1)
        lone = [None, None]
        for h in range(2):  # lone last ACT tile folds in the tail
            if zpend[h] is not None:
                lone[h] = zpend[h][0]

        # ---- tail: per-half combine + partition-reduce via PE transpose ----
        for h in range(2):
            io = h * FDI
            acc = t_zA[h][na[h] % 2]
            nc.vector.tensor_max(t_zfin[:, io : io + FDI], acc[:, 0], acc[:, 1])
            nc.vector.tensor_max(t_zfin[:, io : io + FDI],
                                 t_zfin[:, io : io + FDI],
                                 t_zD[nd[h] % 2][:, io : io + FDI])
            if lone[h] is not None:
                nc.vector.tensor_max(t_zfin[:, io : io + FDI],
                                     t_zfin[:, io : io + FDI], lone[h][:, 0])
            ptr = psum.tile([P, MTI // 2, P], BF16, tag="pcross", name="ptr")
            for c in range(MTI // 2):
                nc.tensor.transpose(
                    ptr[:, c], t_zfin[:, io + c * P : io + (c + 1) * P],
                    t_ident[:])
            q = MTI // 4
            for r in range(2):
                nc.vector.tensor_reduce(
                    t_gmax[:, h * 2 * q + r * q : h * 2 * q + (r + 1) * q],
                    ptr[:, r * q : (r + 1) * q], axis=mybir.AxisListType.X,
                    op=mybir.AluOpType.max)
            nc.sync.dma_start(d_gmax[:, h * 2 * q : (h + 1) * 2 * q],
                              t_gmax[:, h * 2 * q : (h + 1) * 2 * q])

    nc.compile()
    return nc


def _prep_inputs(cluster1: np.ndarray, cluster2: np.ndarray):
    """Host-side sharding + operand layout prep."""
    c2b = cluster2.astype(BF16_NP)
    c2bT = np.ascontiguousarray(c2b.T)                       # [128, 8192] bf16
    sq2 = (c2b.astype(np.float32) ** 2).sum(axis=1)          # [8192] fp32
    sq2neg_h = []
    c2bT_h = []
    for h in range(J_HALVES):
        s = (-sq2[h * NJ : (h + 1) * NJ]).reshape(NJT, P).T
        sq2neg_h.append(np.ascontiguousarray(s).astype(np.float32))
        c2bT_h.append(np.ascontiguousarray(c2bT[:, h * NJ : (h + 1) * NJ]))

    c1bT_g = []
    for g in range(I_GROUPS):
        c1s = cluster1[g * NI : (g + 1) * NI]
        c1bT_g.append(np.ascontiguousarray((2.0 * c1s).astype(BF16_NP).T))

    in_maps = []
    for c in range(N_CORES):
        g, h = c % I_GROUPS, c // I_GROUPS
        in_maps.append({
            "c1bT": c1bT_g[g],
            "c2bT": c2bT_h[h],
            "sq2neg": sq2neg_h[h],
        })
    return in_maps


def _finish(results, cluster1, cluster2) -> np.float32:
    """Combine per-core partials + host-side O(N*D) stats (fp64)."""
    c1 = np.asarray(cluster1, np.float64)
    c2 = np.asarray(cluster2, np.float64)

    # distance term: d2_i = |c1_i|^2 - max_j(2<c1,c2> - |c2_j|^2)
    # (cross/bias computed on device from bf16-rounded operands)
    sq1 = (c1 * c1).sum(axis=1)  # [8192]
    dist_sum = 0.0
    for g in range(I_GROUPS):
        gm0 = np.asarray(results[g]["gmax"], np.float64)            # [128, 16]
        gm1 = np.asarray(results[g + I_GROUPS]["gmax"], np.float64)
        # column b covers i-half b//8, block b%8: i_local = 1024*(b//8)
        #   + 128*(b%8) + p  == 128*b + p  (b ordered h-major == block-major)
        gm = np.maximum(gm0, gm1)                  # [p, b]
        gmax_rows = gm.T.reshape(NI)               # [2048] in i_local order
        dist_sum += (sq1[g * NI : (g + 1) * NI] - gmax_rows).sum()
    dist = dist_sum / N1

    m1 = c1.mean(axis=0)
    m2 = c2.mean(axis=0)
    mean_loss = ((m1 - m2) ** 2).mean()
    var = (c1 * c1).mean(axis=0) - m1 ** 2
    disp = np.maximum(MIN_VARIANCE - var, 0.0).mean()
    return np.float32(mean_loss + dist + disp)


def _run(inputs, trace=False, **kwargs):
    """Run on the 8 NeuronCores. Returns (loss_scalar, BassKernelResults)."""
    if "nc" not in _cached:
        _cached["nc"] = _build_program()
    nc = _cached["nc"]
    c1 = np.asarray(inputs["cluster1"], np.float32)
    c2 = np.asarray(inputs["cluster2"], np.float32)
    in_maps = _prep_inputs(c1, c2)
    res = run_bass_kernel_spmd(nc, in_maps, list(range(N_CORES)), trace=trace,
                               **kwargs)
    loss = _finish(res.results, c1, c2)
    return loss, res


def kernel(cluster1: np.ndarray, cluster2: np.ndarray) -> np.ndarray:
    loss, _ = _run({"cluster1": cluster1, "cluster2": cluster2})
    return np.asarray(loss, dtype=np.float32)


# revision 12
# speedup vs baseline: 1.0207x; 1.0207x over previous
"""Trainium2 Bass kernel for nn_CustomLoss_45449343926664 (retrieval_knn).

loss = mse(mean(c1), mean(c2))
     + mean_i min_j ||c1_i - c2_j||^2
     + mean_k relu(0.1 - var(c1)_k)

Device computes the dominant term: per-row max_j(2<c1_i,c2_j> - |c2_j|^2)
(min-distance via d2 = |c1_i|^2 - that max). The tiny O(N*D) stats
(means / variances / |c1_i|^2) are host-side in fp64, fused into the
final scalar in _finish.

Sharding (8 cores = 4 i-groups x 2 j-halves): core c owns c1 rows
[2048*(c%4), 2048*(c%4+1)) and c2 rows [4096*(c//4), 4096*(c//4+1)).

Per core: 32 j-tiles of 128, each computed as two [128 j, 1024 i] PSUM
units (2 banks each, psum pool bufs=4 so the PE runs ahead of the
drains). Cross matmuls in "j-on-partitions" orientation (c2bT tile
stationary, c1bT moving, bf16, c1 pre-scaled by 2). The 64 units drain
through the only two engines with PSUM read ports, balanced to ~equal
busy time:

  - 17 units: DVE fused scalar_tensor_tensor drain
        zD' = max(psum + bias_j, zD)       (1 pass, per-i-half ping-pong)
  - 47 units: ACT activation(Identity, bias_j) -> bf16 z tiles; pairs
    of same-i-half z tiles fold via one DVE bf16 tensor_tensor max (2x
    mode, [128, 2048]) into per-half ping-pong accumulators.

Tail (per i-half, pipelined): max(accA halves) -> max(.., zD) -> 8 PE
transposes -> 3D reduce_max -> gmax[p, b] for query i = 128*b + p
(+ 2048*(c%4)). Host combines the two j-halves and finishes in fp64.
"""
import os
import sys

import numpy as np
import ml_dtypes

if os.path.isdir("/opt/trn_rl_repo") and "/opt/trn_rl_repo" not in sys.path:
    sys.path.insert(0, "/opt/trn_rl_repo")

from contextlib import ExitStack

import concourse.bass as bass
import concourse.tile as tile
from concourse import bacc, mybir
from concourse.bass_utils import run_bass_kernel_spmd
from concourse.masks import make_identity

F32 = mybir.dt.float32
BF16 = mybir.dt.bfloat16
BF16_NP = ml_dtypes.bfloat16
NEG_BIG = -3.0e38

N_CORES = 8
N1 = 8192            # cluster1 rows (total)
N2 = 8192            # cluster2 rows
D = 128              # feature dim = partition count
P = 128
I_GROUPS = 4
J_HALVES = 2
NI = N1 // I_GROUPS  # 2048 c1 rows per core
NJ = N2 // J_HALVES  # 4096 c2 rows per core
NJT = NJ // P        # 32 j-tiles of 128
MTI = NI // P        # 16 i-blocks of 128 (for the transpose tail)
FDI = 1024           # i-extent per PSUM unit (2 banks)
NU = NJT * 2         # 64 drain units (j-tile x i-half)
MM_SPLIT = 2         # matmuls per unit (one PSUM bank each)

# units on the zD path (18 of 64, none in the final stretch); the first
# of each i-half is a seed executed on ACT (Identity+bias straight into
# zD), so DVE runs 16 fused stt drains and ACT 48 activations.
DVE_UNITS = frozenset({0, 6, 12, 18, 24, 30, 36, 42, 48, 54,
                       3, 9, 15, 21, 27, 33, 39, 45})
MIN_VARIANCE = 0.1

_cached = {}


def _build_program():
    """Build + compile the single-core SPMD program (same for all cores)."""
    nc = bacc.Bacc(
        "TRN2",
        target_bir_lowering=False,
        debug=False,
        enable_asserts=False,
        num_devices=N_CORES,
    )

    d_c1bT = nc.dram_tensor("c1bT", [D, NI], BF16, kind="ExternalInput").ap()
    d_c2bT = nc.dram_tensor("c2bT", [D, NJ], BF16, kind="ExternalInput").ap()
    d_sq2neg = nc.dram_tensor("sq2neg", [P, NJT], F32, kind="ExternalInput").ap()

    d_gmax = nc.dram_tensor("gmax", [P, MTI], F32, kind="ExternalOutput").ap()

    with tile.TileContext(nc) as tc, ExitStack() as ctx:
        const = ctx.enter_context(tc.tile_pool(name="const", bufs=1))
        zpool = [ctx.enter_context(tc.tile_pool(name=f"zp{h}", bufs=3))
                 for h in range(2)]
        psum = ctx.enter_context(tc.tile_pool(name="psum", bufs=4, space="PSUM"))

        t_c1bT = const.tile([P, NI], BF16)
        t_c2bT = const.tile([P, NJ], BF16)
        t_sq2neg = const.tile([P, NJT], F32)
        # per-i-half fold accumulators (ping-pong) + DVE-direct accumulators
        t_zA = [[const.tile([P, 2, FDI], BF16, name=f"zA{h}_{i}")
                 for i in range(2)] for h in range(2)]
        t_zD = [const.tile([P, NI], BF16, name=f"zD{i}") for i in range(2)]
        t_zfin = const.tile([P, NI], BF16)
        t_gmax = const.tile([P, MTI], F32)
        t_ident = const.tile([P, P], BF16)
        t_dummy = const.tile([P, 1], F32)

        # ---- input DMAs: ALL on the sync ring, strict FIFO priority.
        # (All rings share the 16 SDMA engines packet-round-robin, so a
        # second ring's bulk load would steal bandwidth from the critical
        # head; one ring with careful ordering is strictly better.)
        nc.sync.dma_start(t_c2bT[:, 0 : 2 * P], d_c2bT[:, 0 : 2 * P])
        nc.sync.dma_start(t_c1bT[:, 0:512], d_c1bT[:, 0:512])
        nc.sync.dma_start(t_c1bT[:, 512:FDI], d_c1bT[:, 512:FDI])
        nc.sync.dma_start(t_sq2neg[:], d_sq2neg)
        nc.sync.dma_start(t_c2bT[:, 2 * P : 8 * P], d_c2bT[:, 2 * P : 8 * P])
        nc.sync.dma_start(t_c1bT[:, FDI:], d_c1bT[:, FDI:])
        nc.sync.dma_start(t_c2bT[:, 8 * P : 20 * P], d_c2bT[:, 8 * P : 20 * P])
        nc.sync.dma_start(t_c2bT[:, 20 * P :], d_c2bT[:, 20 * P :])

        # warm the ACT function table (load ~1.3us) before the first drain
        nc.vector.memset(t_dummy[:], 1.0)
        nc.scalar.activation(t_dummy[:], t_dummy[:],
                             mybir.ActivationFunctionType.Identity, bias=0.0)

        # identity (for the PE transpose tail) on gpsimd
        make_identity(nc, t_ident[:])

        # ramp the PE p-state while the first inputs land (takes a pool slot
        # whose WAW release happens naturally when the ring wraps)
        pwarm = psum.tile([P, FDI], F32, tag="pcross", name="pwarm")
        for w in range(6):
            nc.tensor.matmul(pwarm[:, :P], t_ident[:], t_ident[:],
                             start=(w == 0), stop=(w == 5))

        # ---- cross matmuls (j on partitions) + dual-engine drain ----
        nd = [0, 0]          # zD ping-pong index per i-half
        na = [0, 0]          # zA ping-pong index per i-half
        zpend = [None, None]  # partially-filled z pair per i-half
        seq = [(t, 0) for t in range(NJT)] + [(t, 1) for t in range(NJT)]
        for u, (t, h) in enumerate(seq):
            pt = psum.tile([P, FDI], F32, tag="pcross", name="pcross")
            lhsT = t_c2bT[:, t * P : (t + 1) * P]
            nmm = MM_SPLIT
            fd = FDI // nmm
            for c in range(nmm):
                nc.tensor.matmul(
                    pt[:, c * fd : (c + 1) * fd],
                    lhsT,
                    t_c1bT[:, h * FDI + c * fd : h * FDI + (c + 1) * fd],
                    start=True,
                    stop=True,
                )
            bias = t_sq2neg[:, t : t + 1]
            if u in DVE_UNITS:
                io = h * FDI
                if nd[h] == 0:
                    # first unit of this half seeds the accumulator straight
                    # from PSUM on the ACT engine (Identity + bias); unit 0
                    # is split in two so the first drain starts right after
                    # the first matmul chunk
                    for lo, hi in ([(0, 512), (512, FDI)] if u == 0
                                   else [(0, FDI)]):
                        nc.scalar.activation(
                            t_zD[1][:, io + lo : io + hi], pt[:, lo:hi],
                            mybir.ActivationFunctionType.Identity,
                            bias=bias, scale=1.0)
                else:
                    nc.vector.scalar_tensor_tensor(
                        out=t_zD[(nd[h] + 1) % 2][:, io : io + FDI],
                        in0=pt[:],
                        scalar=bias,
                        in1=t_zD[nd[h] % 2][:, io : io + FDI],
                        op0=mybir.AluOpType.add,
                        op1=mybir.AluOpType.max,
                    )
                nd[h] += 1
            else:
                if zpend[h] is None:
                    zpend[h] = (zpool[h].tile([P, 2, FDI], BF16, name=f"z{h}"), 0)
                zt, m = zpend[h]
                nc.scalar.activation(
                    zt[:, m], pt[:], mybir.ActivationFunctionType.Identity,
                    bias=bias, scale=1.0,
                )
                if m == 1:
                    # one bf16 2x tensor_max folds the pair into the two
                    # independent slots of this half's accumulator
                    if na[h] == 0:
                        nc.vector.tensor_copy(t_zA[h][1][:], zt[:])
                    else:
                        nc.vector.tensor_max(t_zA[h][(na[h] + 1) % 2][:],
                                             t_zA[h][na[h] % 2][:], zt[:])
                    na[h] += 1
                    zpend[h] = None
                else:
                    zpend[h] = (zt, 1)
        lone = [None, None]
        for h in range(2):  # lone last ACT tile folds in the tail
            if zpend[h] is not None:
                lone[h] = zpend[h][0]

        # ---- tail: per-half combine + partition-reduce via PE transpose ----
        for h in range(2):
            io = h * FDI
            acc = t_zA[h][na[h] % 2]
            nc.vector.tensor_max(t_zfin[:, io : io + FDI], acc[:, 0], acc[:, 1])
            nc.vector.tensor_max(t_zfin[:, io : io + FDI],
                                 t_zfin[:, io : io + FDI],
                                 t_zD[nd[h] % 2][:, io : io + FDI])
            if lone[h] is not None:
                nc.vector.tensor_max(t_zfin[:, io : io + FDI],
                                     t_zfin[:, io : io + FDI], lone[h][:, 0])
            ptr = psum.tile([P, MTI // 2, P], BF16, tag="pcross", name="ptr")
            for c in range(MTI // 2):
                nc.tensor.transpose(
                    ptr[:, c], t_zfin[:, io + c * P : io + (c + 1) * P],
                    t_ident[:])
            q = MTI // 4
            for r in range(2):
                nc.vector.tensor_reduce(
                    t_gmax[:, h * 2 * q + r * q : h * 2 * q + (r + 1) * q],
                    ptr[:, r * q : (r + 1) * q], axis=mybir.AxisListType.X,
                    op=mybir.AluOpType.max)
            nc.scalar.dma_start(d_gmax[:, h * 2 * q : (h + 1) * 2 * q],
                                t_gmax[:, h * 2 * q : (h + 1) * 2 * q])

    nc.compile()
    return nc


def _prep_inputs(cluster1: np.ndarray, cluster2: np.ndarray):
    """Host-side sharding + operand layout prep."""
    c2b = cluster2.astype(BF16_NP)
    c2bT = np.ascontiguousarray(c2b.T)                       # [128, 8192] bf16
    sq2 = (c2b.astype(np.float32) ** 2).sum(axis=1)          # [8192] fp32
    sq2neg_h = []
    c2bT_h = []
    for h in range(J_HALVES):
        s = (-sq2[h * NJ : (h + 1) * NJ]).reshape(NJT, P).T
        sq2neg_h.append(np.ascontiguousarray(s).astype(np.float32))
        c2bT_h.append(np.ascontiguousarray(c2bT[:, h * NJ : (h + 1) * NJ]))

    c1bT_g = []
    for g in range(I_GROUPS):
        c1s = cluster1[g * NI : (g + 1) * NI]
        c1bT_g.append(np.ascontiguousarray((2.0 * c1s).astype(BF16_NP).T))

    in_maps = []
    for c in range(N_CORES):
        g, h = c % I_GROUPS, c // I_GROUPS
        in_maps.append({
            "c1bT": c1bT_g[g],
            "c2bT": c2bT_h[h],
            "sq2neg": sq2neg_h[h],
        })
    return in_maps


def _finish(results, cluster1, cluster2) -> np.float32:
    """Combine per-core partials + host-side O(N*D) stats (fp64)."""
    c1 = np.asarray(cluster1, np.float64)
    c2 = np.asarray(cluster2, np.float64)

    # distance term: d2_i = |c1_i|^2 - max_j(2<c1,c2> - |c2_j|^2)
    # (cross/bias computed on device from bf16-rounded operands)
    sq1 = (c1 * c1).sum(axis=1)  # [8192]
    dist_sum = 0.0
    for g in range(I_GROUPS):
        gm0 = np.asarray(results[g]["gmax"], np.float64)            # [128, 16]
        gm1 = np.asarray(results[g + I_GROUPS]["gmax"], np.float64)
        # column b covers i-half b//8, block b%8: i_local = 1024*(b//8)
        #   + 128*(b%8) + p  == 128*b + p  (b ordered h-major == block-major)
        gm = np.maximum(gm0, gm1)                  # [p, b]
        gmax_rows = gm.T.reshape(NI)               # [2048] in i_local order
        dist_sum += (sq1[g * NI : (g + 1) * NI] - gmax_rows).sum()
    dist = dist_sum / N1

    m1 = c1.mean(axis=0)
    m2 = c2.mean(axis=0)
    mean_loss = ((m1 - m2) ** 2).mean()
    var = (c1 * c1).mean(axis=0) - m1 ** 2
    disp = np.maximum(MIN_VARIANCE - var, 0.0).mean()
    return np.float32(mean_loss + dist + disp)


def _run(inputs, trace=False, **kwargs):
    """Run on the 8 NeuronCores. Returns (loss_scalar, BassKernelResults)."""
    if "nc" not in _cached:
        _cached["nc"] = _build_program()
    nc = _cached["nc"]
    c1 = np.asarray(inputs["cluster1"], np.float32)
    c2 = np.asarray(inputs["cluster2"], np.float32)
    in_maps = _prep_inputs(c1, c2)
    res = run_bass_kernel_spmd(nc, in_maps, list(range(N_CORES)), trace=trace,
                               **kwargs)
    loss = _finish(res.results, c1, c2)
    return loss, res


def kernel(cluster1: np.ndarray, cluster2: np.ndarray) -> np.ndarray:
    loss, _ = _run({"cluster1": cluster1, "cluster2": cluster2})
    return np.asarray(loss, dtype=np.float32)
